# revision 5
# baseline (speedup 1.0000x reference)
"""CogVLM VisionExpert MLP (moe routing) on 8 trn2 NeuronCores.

Strategy:
  - Host computes the vision/language routing mask in numpy and permutes
    tokens by expert. Each token goes through exactly ONE expert (the
    reference computes both and selects; routing halves the matmul work).
  - Default sharding (expert-affinity DP4 x TP2): four 2-core tensor-parallel
    pairs, each pair owns one expert and a contiguous slice of that expert's
    tokens. TP halves split I=11008 -> 5504 per core. Host sums each pair's
    two partial outputs and un-permutes.
  - Matmuls run in fp8e4 (e4m3) DoubleRow perf mode: one DR instruction
    processes K=256 contraction rows at 0.5 cycles/output-column, 4x the
    per-row bf16 rate (HW-verified). To keep accuracy, every operand is
    split into fp8 hi + fp8 lo (error feedback, matched scales so all
    products accumulate in one PSUM group) and each logical matmul computes
    3 products: hi*hi + lo*hi + hi*lo, dropping the O(2^-8) lo*lo term.
    Net PE cost = 0.75x of a bf16 kernel; end-to-end rel err ~2e-3 (better
    than bf16's 4e-3 since the residual pairs carry ~8 mantissa bits).
  - The intermediate a = silu(g)*u is split hi/lo on device: scalar engine
    casts a*SA -> fp8 (hi), DVE computes lo = a*SA - hi with a fused
    scalar_tensor_tensor. Descales fold into activation scale params.
  - Fallback (TP8 over I, all tokens on every core, bf16) when the expert
    split is too skewed for affinity pairs to balance.

Per-core layouts (everything lands in SBUF with 128-partition-first shape):
  xh/xl [NKP, 128, 2, T]   fp8  xh[kp, p, i, t]  = Xq[t, (2kp+i)*128+p]
  g/u   [ni, 128, NK*128]  fp8  g[it, p, k*128+c] = (Wg_sh*SW)[k*128+p, it*128+c]
  d     [NK, 128, ni*128]  fp8  d[h, p, i*128+c]  = (Wd_sh*SWD)[i*128+p, h*128+c]
  yt    [NK, 128, T]       f32  yt[h, p, t]       = Y_part[t, h*128+p] * SA*SWD
DoubleRow operands come from flat-tile slices: stationary w[:, 2k:2k+2, :]
-> [128, 2, 128], moving x[kp][:, :, c0:c1] / a[:, 2i:2i+2, c0:c1]
-> [128, 2, w<=256].
"""
import os
import numpy as np
import ml_dtypes

import concourse.bacc as bacc
import concourse.mybir as mybir
import concourse.tile as tile
from concourse.bass_utils import run_bass_kernel_spmd
from concourse.bass_interp import get_hw_module

bf16 = ml_dtypes.bfloat16
f8 = ml_dtypes.float8_e4m3
B, S, H, I = 2, 2048, 4096, 11008
NCORES = 8
NK = H // 128                # 32 h tiles
NKP = NK // 2                # 16 DR pairs over H
VISION_TOKEN_TYPE = 1

# TP8 fallback geometry (bf16)
ISH8 = I // 8                # 1376 real icols per core
NI8 = (ISH8 + 127) // 128    # 11 tiles (padded to 1408)
TSUB8 = 512
# expert-affinity DP4 x TP2 geometry (fp8 S4)
ISH2 = I // 2                # 5504 icols per core
NI2 = ISH2 // 128            # 43 tiles, exact
NIP = NI2 // 2               # 21 DR pairs over I-shard (+1 odd tile)
TSUB = 256                   # DR moving-dim chunk (rhs free = 2*256 = 512)

# fp8 scales: data sigma -> fp8 sweet spot; residuals land at sigma*2^-5.5,
# still well inside e4m3's normal range, so hi and lo share one scale and
# all 3 products accumulate in a single PSUM group.
SX = 16.0                    # x ~ N(0,1)
SW = 1024.0                  # Wg/Wu ~ N(0, 1/H)
SWD = 1024.0                 # Wd ~ N(0, 1/I)
SA = 4.0                     # a = silu(g)*u, |a|_max ~ 25
S_GU = SX * SW               # scale of g/u in PSUM
DESCALE_SILU = 1.0 / S_GU
CAST_A = SA / S_GU
DESCALE_Y = 1.0 / (SA * SWD)

FP32 = mybir.dt.float32
BF16 = mybir.dt.bfloat16
FP8 = mybir.dt.float8e4
DR = mybir.MatmulPerfMode.DoubleRow

_nc_cache = {}

# observability for test harnesses (not used by grading)
last_results = None
last_run = None  # (nc, in_maps) of the most recent kernel() call


def _plan_blocks(n, tblk):
    """Split a token count into blocks of <= tblk+128 (weights re-stream once
    per block). A remainder <=128 is folded into the last block."""
    out = []
    t = 0
    while n - t >= tblk:
        out.append((t, tblk))
        t += tblk
    r = n - t
    if r > 0:
        if r <= 128 and out:
            t0, tc = out[-1]
            out[-1] = (t0, tc + r)
        else:
            out.append((t, r))
    return out


def _subs(tcols, tsub):
    """Split a block into balanced matmul moving-dim pieces (<= tsub each)."""
    nsub = (tcols + tsub - 1) // tsub
    base, rem = divmod(tcols, nsub)
    out = []
    c = 0
    for s in range(nsub):
        w = base + (1 if s < rem else 0)
        out.append((c, w))
        c += w
    return out


# ---------------------------------------------------------------------------
# fp8 S4 affinity kernel (DP4 x TP2)
# ---------------------------------------------------------------------------

def _build_s4(cap):
    """Expert-affinity per-core program: swiglu MLP over `cap` tokens with
    the core's I/2 weight shard, fp8 DoubleRow 3-product matmuls."""
    ni = NI2
    blocks = _plan_blocks(cap, 512)
    nc = bacc.Bacc("TRN2", target_bir_lowering=False, debug=False,
                   num_devices=NCORES)

    xh_d = nc.dram_tensor("xh", [NKP, 128, 2, cap], FP8, kind="ExternalInput")
    xl_d = nc.dram_tensor("xl", [NKP, 128, 2, cap], FP8, kind="ExternalInput")
    gh_d = nc.dram_tensor("gh", [ni, 128, NK, 128], FP8, kind="ExternalInput")
    gl_d = nc.dram_tensor("gl", [ni, 128, NK, 128], FP8, kind="ExternalInput")
    uh_d = nc.dram_tensor("uh", [ni, 128, NK, 128], FP8, kind="ExternalInput")
    ul_d = nc.dram_tensor("ul", [ni, 128, NK, 128], FP8, kind="ExternalInput")
    dh_d = nc.dram_tensor("dh", [NK, 128, ni, 128], FP8, kind="ExternalInput")
    dl_d = nc.dram_tensor("dl", [NK, 128, ni, 128], FP8, kind="ExternalInput")
    yt_d = nc.dram_tensor("yt", [NK, 128, cap], FP32, kind="ExternalOutput")

    with tile.TileContext(nc) as tc_:
        with (
            tc_.tile_pool(name="xp", bufs=1) as xp,
            tc_.tile_pool(name="apool", bufs=2) as apool,
            tc_.tile_pool(name="wgu", bufs=2) as wgu,
            tc_.tile_pool(name="wdp", bufs=2) as wdp,
            tc_.tile_pool(name="sp", bufs=2) as sp,
            tc_.tile_pool(name="yp", bufs=2) as yp,
            tc_.tile_pool(name="pg", bufs=2, space="PSUM") as pgp,
            tc_.tile_pool(name="pu", bufs=2, space="PSUM") as pup,
            tc_.tile_pool(name="py", bufs=4, space="PSUM") as pyp,
        ):
            for bi, (t0, tcols) in enumerate(blocks):
                subs = _subs(tcols, TSUB)
                t1 = t0 + tcols
                # ---- stage the block's inputs ----
                wgh0 = wgu.tile([128, NK, 128], FP8, tag="wgh", name="wgh0")
                wgl0 = wgu.tile([128, NK, 128], FP8, tag="wgl", name="wgl0")
                wuh0 = wgu.tile([128, NK, 128], FP8, tag="wuh", name="wuh0")
                wul0 = wgu.tile([128, NK, 128], FP8, tag="wul", name="wul0")
                xh_sb = [xp.tile([128, 2, tcols], FP8, tag=f"xh{k}",
                                 name=f"xh{k}") for k in range(NKP)]
                xl_sb = [xp.tile([128, 2, tcols], FP8, tag=f"xl{k}",
                                 name=f"xl{k}") for k in range(NKP)]
                if bi == 0:
                    # startup is latency-critical: first weight tile spread
                    # over 4 queues, x pair tiles interleaved right behind
                    q4 = NK // 4
                    nc.sync.dma_start(xh_sb[0][:], xh_d.ap()[0, :, :, t0:t1])
                    for q in range(4):
                        nc.sync.dma_start(wgh0[:, q * q4:(q + 1) * q4, :],
                                          gh_d.ap()[0, :, q * q4:(q + 1) * q4, :])
                    for k in range(1, NKP):
                        nc.sync.dma_start(xh_sb[k][:], xh_d.ap()[k, :, :, t0:t1])
                    for q in range(4):
                        nc.sync.dma_start(wgl0[:, q * q4:(q + 1) * q4, :],
                                          gl_d.ap()[0, :, q * q4:(q + 1) * q4, :])
                    for k in range(NKP):
                        nc.sync.dma_start(xl_sb[k][:], xl_d.ap()[k, :, :, t0:t1])
                    nc.sync.dma_start(wuh0[:], uh_d.ap()[0])
                    nc.sync.dma_start(wul0[:], ul_d.ap()[0])
                else:
                    nc.sync.dma_start(wgh0[:], gh_d.ap()[0])
                    nc.sync.dma_start(wgl0[:], gl_d.ap()[0])
                    nc.sync.dma_start(wuh0[:], uh_d.ap()[0])
                    nc.sync.dma_start(wul0[:], ul_d.ap()[0])
                    for k in range(NKP):
                        nc.sync.dma_start(xh_sb[k][:], xh_d.ap()[k, :, :, t0:t1])
                    for k in range(NKP):
                        nc.sync.dma_start(xl_sb[k][:], xl_d.ap()[k, :, :, t0:t1])

                ah_sb = apool.tile([128, ni, tcols], FP8, tag="ah", name="ah")
                al_sb = apool.tile([128, ni, tcols], FP8, tag="al", name="al")

                # ---- gate/up + silu*up + a hi/lo split ----
                for it in range(ni):
                    if it == 0:
                        wgh, wgl, wuh, wul = wgh0, wgl0, wuh0, wul0
                    else:
                        wgh = wgu.tile([128, NK, 128], FP8, tag="wgh")
                        wgl = wgu.tile([128, NK, 128], FP8, tag="wgl")
                        wuh = wgu.tile([128, NK, 128], FP8, tag="wuh")
                        wul = wgu.tile([128, NK, 128], FP8, tag="wul")
                        nc.sync.dma_start(wgh[:], gh_d.ap()[it])
                        nc.sync.dma_start(wgl[:], gl_d.ap()[it])
                        nc.sync.dma_start(wuh[:], uh_d.ap()[it])
                        nc.sync.dma_start(wul[:], ul_d.ap()[it])
                    for (c0, w) in subs:
                        c1 = c0 + w
                        pg = pgp.tile([128, w], FP32, tag="pg")
                        pu = pup.tile([128, w], FP32, tag="pu")
                        for ps, (wh, wl) in ((pg, (wgh, wgl)), (pu, (wuh, wul))):
                            prods = ((wh, xh_sb), (wl, xh_sb), (wh, xl_sb))
                            for pi, (wt, xs_) in enumerate(prods):
                                for kp in range(NKP):
                                    nc.tensor.matmul(
                                        ps[:], wt[:, 2 * kp:2 * kp + 2, :],
                                        xs_[kp][:, :, c0:c1],
                                        start=(pi == 0 and kp == 0),
                                        stop=(pi == 2 and kp == NKP - 1),
                                        perf_mode=DR)
                        sil = sp.tile([128, w], FP32, tag="sil")
                        nc.scalar.activation(sil[:], pg[:],
                                             mybir.ActivationFunctionType.Silu,
                                             scale=DESCALE_SILU)
                        araw = sp.tile([128, w], FP32, tag="araw")
                        nc.vector.tensor_mul(araw[:], sil[:], pu[:])
                        nc.scalar.activation(ah_sb[:, it, c0:c1], araw[:],
                                             mybir.ActivationFunctionType.Copy,
                                             scale=CAST_A)
                        nc.vector.scalar_tensor_tensor(
                            al_sb[:, it, c0:c1], araw[:], CAST_A,
                            ah_sb[:, it, c0:c1],
                            mybir.AluOpType.mult, mybir.AluOpType.subtract)

                # ---- down projection ----
                for h in range(NK):
                    wdh = wdp.tile([128, ni, 128], FP8, tag="wdh")
                    wdl = wdp.tile([128, ni, 128], FP8, tag="wdl")
                    nc.sync.dma_start(wdh[:], dh_d.ap()[h])
                    nc.sync.dma_start(wdl[:], dl_d.ap()[h])
                    for (c0, w) in subs:
                        c1 = c0 + w
                        py = pyp.tile([128, w], FP32, tag="py")
                        prods = ((wdh, ah_sb), (wdl, ah_sb), (wdh, al_sb))
                        for pi, (wt, at_) in enumerate(prods):
                            for ip in range(NIP):
                                nc.tensor.matmul(
                                    py[:], wt[:, 2 * ip:2 * ip + 2, :],
                                    at_[:, 2 * ip:2 * ip + 2, c0:c1],
                                    start=(pi == 0 and ip == 0), stop=False,
                                    perf_mode=DR)
                            # odd 43rd tile: plain fp8 matmul
                            nc.tensor.matmul(
                                py[:], wt[:, NI2 - 1, :],
                                at_[:, NI2 - 1, c0:c1],
                                start=False, stop=(pi == 2))
                        y_sb = yp.tile([128, w], FP32, tag="y")
                        nc.scalar.activation(y_sb[:], py[:],
                                             mybir.ActivationFunctionType.Copy,
                                             scale=DESCALE_Y)
                        nc.sync.dma_start(yt_d.ap()[h, :, t0 + c0:t0 + c1],
                                          y_sb[:])

    nc.compile()
    nc.m = get_hw_module(nc.m)
    return nc


def _split8(a, s):
    """fp32 array -> (hi, lo) fp8 e4m3 at matched scale s."""
    sc = np.asarray(a, dtype=np.float32) * s
    hi = sc.astype(f8)
    lo = (sc - hi.astype(np.float32)).astype(f8)
    return hi, lo


def _tile_gu8(Wsc):
    """scaled [H, ish] fp8 -> [ni, 128, NK, 128] column-shard tiles."""
    ish = Wsc.shape[1]
    ni = ish // 128
    t = Wsc.reshape(NK, 128, ni, 128).transpose(2, 1, 0, 3)
    return np.ascontiguousarray(t)


def _tile_d8(Wsc):
    """scaled [ish, H] fp8 -> [NK, 128, ni, 128] row-shard tiles."""
    ish = Wsc.shape[0]
    ni = ish // 128
    t = Wsc.reshape(ni, 128, NK, 128).transpose(2, 1, 0, 3)
    return np.ascontiguousarray(t)


def _tile_x8(Xq, cap):
    """[cap, H] fp8 -> [NKP, 128, 2, cap] DR pair layout."""
    t = np.ascontiguousarray(Xq.T).reshape(NKP, 2, 128, cap).transpose(0, 2, 1, 3)
    return np.ascontiguousarray(t)


# ---------------------------------------------------------------------------
# bf16 TP8 fallback (unchanged from baseline)
# ---------------------------------------------------------------------------

def _build_tp8(nl, nv):
    blocks = [("l", t0, tc) for (t0, tc) in _plan_blocks(nl, 1024)]
    blocks += [("v", nl + t0, tc) for (t0, tc) in _plan_blocks(nv, 1024)]
    return _build_bf16(nl + nv, NI8,
                       {"l": ("gl", "ul", "dl"), "v": ("gv", "uv", "dv")},
                       blocks, a_bufs=2, wd_bufs=3, y_bufs=4)


def _build_bf16(Tt, ni, weight_sets, blocks, a_bufs, wd_bufs, y_bufs):
    nc = bacc.Bacc("TRN2", target_bir_lowering=False, debug=False,
                   num_devices=NCORES)

    xt_d = nc.dram_tensor("xt", [NK, 128, Tt], BF16, kind="ExternalInput")
    w_d = {}
    for key, (gn, un, dn) in weight_sets.items():
        w_d[key] = (
            nc.dram_tensor(gn, [ni, 128, NK * 128], BF16, kind="ExternalInput"),
            nc.dram_tensor(un, [ni, 128, NK * 128], BF16, kind="ExternalInput"),
            nc.dram_tensor(dn, [NK, 128, ni * 128], BF16, kind="ExternalInput"),
        )
    yt_d = nc.dram_tensor("yt", [NK, 128, Tt], FP32, kind="ExternalOutput")

    with tile.TileContext(nc) as tc_:
        with (
            tc_.tile_pool(name="xp", bufs=1) as xp,
            tc_.tile_pool(name="apool", bufs=a_bufs) as apool,
            tc_.tile_pool(name="wgu", bufs=2) as wgu,
            tc_.tile_pool(name="wdp", bufs=wd_bufs) as wdp,
            tc_.tile_pool(name="sp", bufs=2) as sp,
            tc_.tile_pool(name="yp", bufs=y_bufs) as yp,
            tc_.tile_pool(name="pg", bufs=2, space="PSUM") as pgp,
            tc_.tile_pool(name="pu", bufs=2, space="PSUM") as pup,
            tc_.tile_pool(name="py", bufs=4, space="PSUM") as pyp,
        ):
            for bi, (key, t0, tcols) in enumerate(blocks):
                g_d, u_d, d_d = w_d[key]
                subs = _subs(tcols, TSUB8)
                wg0 = wgu.tile([128, NK * 128], BF16, tag="wg", name="wg0")
                wu0 = wgu.tile([128, NK * 128], BF16, tag="wu", name="wu0")
                x_sb = [xp.tile([128, tcols], BF16, tag=f"x{k}", name=f"xsb{k}")
                        for k in range(NK)]
                wgu1 = None
                if bi == 0:
                    nc.sync.dma_start(x_sb[0][:], xt_d.ap()[0, :, t0:t0 + tcols])
                    q4 = NK * 128 // 4
                    for q in range(4):
                        nc.sync.dma_start(wg0[:, q * q4:(q + 1) * q4],
                                          g_d.ap()[0, :, q * q4:(q + 1) * q4])
                    for q in range(4):
                        nc.sync.dma_start(wu0[:, q * q4:(q + 1) * q4],
                                          u_d.ap()[0, :, q * q4:(q + 1) * q4])
                    chunks = []
                    if ni > 1:
                        wg1 = wgu.tile([128, NK * 128], BF16, tag="wg", name="wg1")
                        wu1 = wgu.tile([128, NK * 128], BF16, tag="wu", name="wu1")
                        wgu1 = (wg1, wu1)
                        chunks = ([(wg1, g_d, q) for q in range(4)]
                                  + [(wu1, u_d, q) for q in range(4)])
                    ci = 0
                    for k in range(1, NK):
                        nc.sync.dma_start(x_sb[k][:], xt_d.ap()[k, :, t0:t0 + tcols])
                        if (k % 4 == 0 or k == NK - 1) and ci < len(chunks):
                            t_, d_, q = chunks[ci]
                            ci += 1
                            nc.sync.dma_start(t_[:, q * q4:(q + 1) * q4],
                                              d_.ap()[1, :, q * q4:(q + 1) * q4])
                    while ci < len(chunks):
                        t_, d_, q = chunks[ci]
                        ci += 1
                        nc.sync.dma_start(t_[:, q * q4:(q + 1) * q4],
                                          d_.ap()[1, :, q * q4:(q + 1) * q4])
                else:
                    nc.sync.dma_start(wg0[:], g_d.ap()[0])
                    nc.sync.dma_start(wu0[:], u_d.ap()[0])
                    for k in range(NK):
                        nc.sync.dma_start(x_sb[k][:], xt_d.ap()[k, :, t0:t0 + tcols])
                a_sb = apool.tile([128, ni, tcols], BF16, tag="a")
                for it in range(ni):
                    if it == 0:
                        wg_sb, wu_sb = wg0, wu0
                    elif it == 1 and wgu1 is not None:
                        wg_sb, wu_sb = wgu1
                    else:
                        wg_sb = wgu.tile([128, NK * 128], BF16, tag="wg")
                        wu_sb = wgu.tile([128, NK * 128], BF16, tag="wu")
                        nc.sync.dma_start(wg_sb[:], g_d.ap()[it])
                        nc.sync.dma_start(wu_sb[:], u_d.ap()[it])
                    for (c0, w) in subs:
                        c1 = c0 + w
                        pg = pgp.tile([128, w], FP32, tag="pg")
                        pu = pup.tile([128, w], FP32, tag="pu")
                        for k in range(NK):
                            nc.tensor.matmul(pg[:], wg_sb[:, k * 128:(k + 1) * 128],
                                             x_sb[k][:, c0:c1],
                                             start=(k == 0), stop=(k == NK - 1))
                        for k in range(NK):
                            nc.tensor.matmul(pu[:], wu_sb[:, k * 128:(k + 1) * 128],
                                             x_sb[k][:, c0:c1],
                                             start=(k == 0), stop=(k == NK - 1))
                        silu_sb = sp.tile([128, w], FP32, tag="silu")
                        nc.scalar.activation(silu_sb[:], pg[:],
                                             mybir.ActivationFunctionType.Silu)
                        nc.vector.tensor_mul(a_sb[:, it, c0:c1], silu_sb[:], pu[:])
                for h in range(NK):
                    wd_sb = wdp.tile([128, ni * 128], BF16, tag="wd")
                    nc.sync.dma_start(wd_sb[:], d_d.ap()[h])
                    for (c0, w) in subs:
                        c1 = c0 + w
                        py = pyp.tile([128, w], FP32, tag="py")
                        for i in range(ni):
                            nc.tensor.matmul(py[:], wd_sb[:, i * 128:(i + 1) * 128],
                                             a_sb[:, i, c0:c1],
                                             start=(i == 0), stop=(i == ni - 1))
                        y_sb = yp.tile([128, w], FP32, tag="y")
                        nc.scalar.copy(y_sb[:], py[:])
                        nc.sync.dma_start(yt_d.ap()[h, :, t0 + c0:t0 + c1], y_sb[:])

    nc.compile()
    nc.m = get_hw_module(nc.m)
    return nc


def _tile_gu(W, c, ish, ni):
    """[H, I] f32 -> per-core [ni, 128, NK*128] bf16 column shard."""
    sh = np.asarray(W, dtype=np.float32)[:, c * ish:(c + 1) * ish].astype(bf16)
    pad = ni * 128 - ish
    if pad:
        sh = np.concatenate([sh, np.zeros((H, pad), dtype=bf16)], axis=1)
    t = sh.reshape(NK, 128, ni, 128).transpose(2, 1, 0, 3)
    return np.ascontiguousarray(t).reshape(ni, 128, NK * 128)


def _tile_d(W, c, ish, ni):
    """[I, H] f32 -> per-core [NK, 128, ni*128] bf16 row shard."""
    sh = np.asarray(W, dtype=np.float32)[c * ish:(c + 1) * ish, :].astype(bf16)
    pad = ni * 128 - ish
    if pad:
        sh = np.concatenate([sh, np.zeros((pad, H), dtype=bf16)], axis=0)
    t = sh.reshape(ni, 128, NK, 128).transpose(2, 1, 0, 3)
    return np.ascontiguousarray(t).reshape(NK, 128, ni * 128)


def _chunks(n, k):
    if k <= 0:
        return []
    base, rem = divmod(n, k)
    out, s = [], 0
    for i in range(k):
        c = base + (1 if i < rem else 0)
        out.append((s, c))
        s += c
    return out


def _affinity_shards(Nl, Nv):
    """4 single-expert token shards for the DP4 x TP2 layout, or None if the
    expert split is too skewed for this to beat TP8."""
    if Nl == 0 or Nv == 0:
        k_l = 4 if Nv == 0 else 0
    else:
        k_l = min(3, max(1, round(4 * Nl / (Nl + Nv))))
    shards = ([("l", s, c) for (s, c) in _chunks(Nl, k_l)]
              + [("v", s, c) for (s, c) in _chunks(Nv, 4 - k_l)])
    if len(shards) != 4 or any(c == 0 for _, _, c in shards):
        return None, 0
    cap = max(c for _, _, c in shards)
    # affinity-fp8 per-core PE work ~ 0.77 * cap * ISH2 bf16-equivalents vs
    # TP8-bf16's (all tokens x 1408 padded icols)
    if 0.77 * cap * ISH2 >= (Nl + Nv) * NI8 * 128:
        return None, 0
    return shards, cap


def kernel(hidden_states, token_type_ids, lang_gate, lang_up, lang_down,
           vis_gate, vis_up, vis_down):
    global last_results, last_run
    x = np.asarray(hidden_states, dtype=np.float32).reshape(B * S, H)
    tt = np.asarray(token_type_ids).reshape(B, S)

    vis = np.zeros((B, S), dtype=bool)
    vis[:, :-1] = (tt[:, :-1] == VISION_TOKEN_TYPE) & (tt[:, 1:] == VISION_TOKEN_TYPE)
    visf = vis.reshape(-1)
    lang_idx = np.flatnonzero(~visf)
    vis_idx = np.flatnonzero(visf)
    Nl, Nv = len(lang_idx), len(vis_idx)
    ew = {"l": (lang_gate, lang_up, lang_down), "v": (vis_gate, vis_up, vis_down)}

    shards, cap = _affinity_shards(Nl, Nv)
    if shards is not None:
        # ---- expert-affinity DP4 x TP2, fp8 DoubleRow S4 ----
        key = ("s4", cap)
        if key not in _nc_cache:
            _nc_cache[key] = _build_s4(cap)
        nc = _nc_cache[key]

        wt = {}  # (expert, tp) -> tiled fp8 hi/lo weights
        for e in set(e for e, _, _ in shards):
            g, u, d = ew[e]
            for tp in range(2):
                c0, c1 = tp * ISH2, (tp + 1) * ISH2
                ghi, glo = _split8(np.asarray(g, np.float32)[:, c0:c1], SW)
                uhi, ulo = _split8(np.asarray(u, np.float32)[:, c0:c1], SW)
                dhi, dlo = _split8(np.asarray(d, np.float32)[c0:c1, :], SWD)
                wt[(e, tp)] = {
                    "gh": _tile_gu8(ghi), "gl": _tile_gu8(glo),
                    "uh": _tile_gu8(uhi), "ul": _tile_gu8(ulo),
                    "dh": _tile_d8(dhi), "dl": _tile_d8(dlo),
                }
        in_maps = [None] * NCORES
        shard_idx = []
        for s, (e, st, cnt) in enumerate(shards):
            idx = (lang_idx if e == "l" else vis_idx)[st:st + cnt]
            shard_idx.append(idx)
            xs = np.zeros((cap, H), dtype=np.float32)
            xs[:cnt] = x[idx]
            xhi, xlo = _split8(xs, SX)
            xh_t = _tile_x8(xhi, cap)
            xl_t = _tile_x8(xlo, cap)
            for tp in range(2):
                m = dict(wt[(e, tp)])
                m["xh"] = xh_t
                m["xl"] = xl_t
                in_maps[2 * s + tp] = m

        trace = bool(int(os.environ.get("KERNEL_TRACE", "0")))
        res = run_bass_kernel_spmd(nc, in_maps, list(range(NCORES)), trace=trace)
        last_results = res
        last_run = (nc, in_maps)

        out_flat = np.empty((B * S, H), dtype=np.float32)
        for s, (e, st, cnt) in enumerate(shards):
            ysum = (res.results[2 * s]["yt"] + res.results[2 * s + 1]["yt"])
            out_flat[shard_idx[s]] = ysum.reshape(H, cap)[:, :cnt].T
        return out_flat.reshape(B, S, H)

    # ---- TP8 fallback: shard I 8 ways, every core runs all tokens (bf16) ----
    Tt = Nl + Nv
    xp_ = np.empty((Tt, H), dtype=np.float32)
    xp_[:Nl] = x[lang_idx]
    xp_[Nl:] = x[vis_idx]
    xt = np.ascontiguousarray(xp_.T.astype(bf16)).reshape(NK, 128, Tt)

    key = ("tp8", Nl, Nv)
    if key not in _nc_cache:
        _nc_cache[key] = _build_tp8(Nl, Nv)
    nc = _nc_cache[key]

    in_maps = []
    for c in range(NCORES):
        in_maps.append({
            "xt": xt,
            "gl": _tile_gu(lang_gate, c, ISH8, NI8),
            "ul": _tile_gu(lang_up, c, ISH8, NI8),
            "dl": _tile_d(lang_down, c, ISH8, NI8),
            "gv": _tile_gu(vis_gate, c, ISH8, NI8),
            "uv": _tile_gu(vis_up, c, ISH8, NI8),
            "dv": _tile_d(vis_down, c, ISH8, NI8),
        })

    trace = bool(int(os.environ.get("KERNEL_TRACE", "0")))
    res = run_bass_kernel_spmd(nc, in_maps, list(range(NCORES)), trace=trace)
    last_results = res
    last_run = (nc, in_maps)

    ysum = np.zeros((NK, 128, Tt), dtype=np.float32)
    for r in res.results:
        ysum += r["yt"]
    yt_full = ysum.reshape(H, Tt)
    out_flat = np.empty((B * S, H), dtype=np.float32)
    out_flat[lang_idx] = yt_full[:, :Nl].T
    out_flat[vis_idx] = yt_full[:, Nl:].T
    return out_flat.reshape(B, S, H)
